# revision 17
# baseline (speedup 1.0000x reference)
"""BiDAF2 attention kernel for Trainium2, 8-core data parallel over batch.

reference (per batch b):
  w1h[s,l] = h[s,:] @ w1_w[l,:] + w1_b[l]
  w2q[t,l] = q[t,:] @ w2_w[l,:] + w2_b[l]
  a[s,t]   = w1h[s,t] + w2q[t,s] + h[s,:]@q[t,:]
  p        = softmax_t(a);  c[s,:] = p[s,:] @ q
  m[s]     = max_t a[s,t];  p2 = softmax_s(m)
  out      = concat([h, c, h*c, (h*p2)*c], axis=-1)

v9 design (cost-model driven; single-core TimelineSim matched HW within 4%):
  - PE p-state: the tensor engine only reaches 2.4 GHz after 3us of
    CONTINUOUS execution, so the kernel is built as one long software
    pipeline: step s in [0,16): A(s) logit block, T(s-1) PE-transpose of
    p, C(s-2) fp8 c-matmul. No DMA transposes anywhere.
  - Host prep (sharding time) ships layout/precision variants only:
    h16 [t,d] f16, hT [d,t] f16, qT [d,t] f16, q8 [t,d] fp8e4m3,
    w1wT [d,l] f16, w2T = 8*w2^T fp8 (the 8x / 0.125 scales bake out).
    uT = qT + w1wT is a DVE add on device; qT8 = qT * 0.125 (fp8).
  - Logits: a = hT^T@uT (fp16) + w2T^T@qT8 (fp8 DoubleRow, 2x PE rate)
    + w1_b via ones-row matmul. Softmax row-sum fused into the exp.
  - p transposed by PE (identity matmul) into a PSUM pool shared with the
    c-accumulator (2+2 banks + 2x2 psA = 8 banks exactly); Act copies
    psum -> pT8 fp8 with scale 64 (keeps small p out of fp8 subnormals);
    the 1/64 is folded into the c epilogue scale.
  - c = pT8^T@q8 in fp8 DoubleRow; epilogue scale r/64, r = 1/Z.
  - p2 = softmax over the 1024 row maxes entirely on-chip via gpsimd
    partition_all_reduce (max, then add) -- no DRAM roundtrip.
  - Output is fp16 [c, h*c, (h*c)*p2]; section 0 (h verbatim) is filled
    host-side from the exact f32 input.
"""

import os
import sys

for _p in ("/opt/trn_rl_repo", "/root/.axon_site/_ro/trn_rl_repo"):
    if os.path.isdir(_p) and _p not in sys.path:
        sys.path.append(_p)

from contextlib import ExitStack

import numpy as np

import concourse.bass as bass
import concourse.tile as tile
from concourse import bacc, bass_isa, mybir
from concourse.bass_utils import run_bass_kernel_spmd

B, L, D = 16, 1024, 768
NCORES = 8
BL = B // NCORES  # batches per core
P = 128
KD = D // P  # 6 d-chunks
NT = L // P  # 8 t-chunks == 8 s-tiles
NSTEP = BL * NT  # 16 pipeline steps
F16 = mybir.dt.float16
F32 = mybir.dt.float32
F8 = mybir.dt.float8e4
EXP = mybir.ActivationFunctionType.Exp
COPY = mybir.ActivationFunctionType.Copy
AX = mybir.AxisListType.X
DR = mybir.MatmulPerfMode.DoubleRow

B_FP8 = True  # phaseB (c = p@q) via fp8 DoubleRow; False = fp16 fallback
RESID = True  # add a scale-4 residual DR pass (p (x) 16*(q - q8)) for accuracy
WIDE_A = False  # single 1024-col A matmuls (PSUM bank-crossing; HW-verify!)
WIDE_C = False  # single 768-col C matmuls
PSCALE = 64.0  # pT8 pre-scale (folded out in the c epilogue)
REPEAT = 1  # benchmarking aid: run the whole body REPEAT times via For_i


def _emit(ctx: ExitStack, tc: tile.TileContext, aps):
    if REPEAT > 1:
        with tc.For_i(0, REPEAT, 1):
            _emit_once(ctx, tc, aps)
    else:
        _emit_once(ctx, tc, aps)


def _emit_once(ctx: ExitStack, tc: tile.TileContext, aps):
    nc = tc.nc
    h, hT, qT, q8, dq8, w1wT, w1b, w2t, w2b, out = aps
    halves = [(0, L)] if WIDE_A else [(0, 512), (512, 1024)]

    const = ctx.enter_context(tc.tile_pool(name="const", bufs=1))
    qTld_p = ctx.enter_context(tc.tile_pool(name="qTld", bufs=2))
    uT_p = ctx.enter_context(tc.tile_pool(name="uT", bufs=2))
    qT8_p = ctx.enter_context(tc.tile_pool(name="qT8", bufs=2))
    hT_p = ctx.enter_context(tc.tile_pool(name="hTt", bufs=2))
    h16_p = ctx.enter_context(tc.tile_pool(name="h16", bufs=2))
    q8_p = ctx.enter_context(tc.tile_pool(name="q8t", bufs=2))
    p16_p = ctx.enter_context(tc.tile_pool(name="p16", bufs=2))
    pT8_p = ctx.enter_context(tc.tile_pool(name="pT8", bufs=4))
    c16_p = ctx.enter_context(tc.tile_pool(name="c16", bufs=8))
    ep_p = ctx.enter_context(tc.tile_pool(name="ep", bufs=4))
    smalls = ctx.enter_context(tc.tile_pool(name="smalls", bufs=1))
    psA = ctx.enter_context(tc.tile_pool(name="psA", bufs=2, space="PSUM"))
    psTC = ctx.enter_context(tc.tile_pool(name="psTC", bufs=2, space="PSUM"))

    # ---- constants ----
    ones1 = const.tile([1, P], F16)
    nc.vector.memset(ones1, 1.0)
    id128 = const.tile([P, P], F16, name="id128")
    nc.vector.memset(id128, 1.0)
    # identity: keep the 1.0 where col == partition (iota = col - p == 0)
    nc.gpsimd.affine_select(
        out=id128, in_=id128, pattern=[[1, P]], base=0, channel_multiplier=-1,
        compare_op=mybir.AluOpType.is_equal, fill=0.0,
    )
    w1b16 = const.tile([1, L], F16)
    nc.gpsimd.dma_start(out=w1b16, in_=w1b[None, :])
    w2b_col = const.tile([P, NT], F32)
    nc.sync.dma_start(out=w2b_col, in_=w2b.rearrange("(c p) -> p c", p=P))
    # Load-queue discipline: ALL latency-critical loads go on sync (SP) in
    # priority order; scalar (Act) issues no DMAs so the exp/copyT/ccCopy
    # chain never stalls behind a 667ns DMA trigger; gpsimd (SWDGE) carries
    # the bulk h16/q8 loads and the cc stores.
    w1wT_t = const.tile([P, KD, L], F16, name="w1wTt")  # [d_part, kd, l]
    w2T = const.tile([P, KD, L], F8, name="w2T")  # [d_part, kd, l]
    w2T_src = w2t.rearrange("(k p) l -> p k l", p=P)

    bdt = F8 if B_FP8 else F16
    uT = {}
    qT8 = {}
    qTld = {}
    dq8t = {}
    hTt = {}
    h16 = {}
    q8t = {}
    negm = {}
    negh = {}
    z_col = {}
    r_col = {}
    p2c = {}

    def prep_loads(b):
        qTld[b] = qTld_p.tile([P, KD, L], F16, tag="qTld", name=f"qTld_{b}")
        uT[b] = uT_p.tile([P, KD, L], F16, tag="uT", name=f"uT_{b}")
        qT8[b] = qT8_p.tile([P, KD, L], F8, tag="qT8", name=f"qT8_{b}")
        hTt[b] = hT_p.tile([P, KD, L], F16, tag="hTt", name=f"hTt_{b}")
        hT_src = hT[b].rearrange("(k p) l -> p k l", p=P)
        for k in range(KD):
            r = slice(k * P, (k + 1) * P)
            if b == 0:
                nc.sync.dma_start(out=w1wT_t[:, k, :], in_=w1wT[r, :])
            nc.sync.dma_start(out=qTld[b][:, k, :], in_=qT[b, r, :])
        h16[b] = h16_p.tile([P, NT, D], F16, tag="h16", name=f"h16_{b}")
        q8t[b] = q8_p.tile([P, NT, D], bdt, tag="q8t", name=f"q8t_{b}")
        if B_FP8 and RESID:
            dq8t[b] = q8_p.tile([P, NT, D], F8, tag="dq8t", name=f"dq8t_{b}")
        h16_src = [h[b, 512 * hf:512 * hf + 512, :].rearrange(
            "(c p) d -> p c d", p=P) for hf in range(2)]
        if b == 0:
            # single-queue priority order: hT quarters / w2T halves / bulk
            # loads interleaved exactly when the A/C stream first needs them
            nc.sync.dma_start(out=hTt[b][:, :, 0:256], in_=hT_src[:, :, 0:256])
            nc.sync.dma_start(out=w2T[:, :, 0:512], in_=w2T_src[:, :, 0:512])
            nc.sync.dma_start(out=hTt[b][:, :, 256:512],
                              in_=hT_src[:, :, 256:512])
            nc.sync.dma_start(out=q8t[b], in_=q8[b].rearrange(
                "(c p) d -> p c d", p=P))
            if B_FP8 and RESID:
                nc.sync.dma_start(out=dq8t[b], in_=dq8[b].rearrange(
                    "(c p) d -> p c d", p=P))
            nc.sync.dma_start(out=hTt[b][:, :, 512:768],
                              in_=hT_src[:, :, 512:768])
            nc.sync.dma_start(out=h16[b][:, 0:4, :], in_=h16_src[0])
            nc.sync.dma_start(out=hTt[b][:, :, 768:L], in_=hT_src[:, :, 768:L])
            nc.sync.dma_start(out=w2T[:, :, 512:L], in_=w2T_src[:, :, 512:L])
            nc.sync.dma_start(out=h16[b][:, 4:8, :], in_=h16_src[1])
        else:
            nc.sync.dma_start(out=hTt[b], in_=hT_src)
            nc.sync.dma_start(out=q8t[b], in_=q8[b].rearrange(
                "(c p) d -> p c d", p=P))
            if B_FP8 and RESID:
                nc.sync.dma_start(out=dq8t[b], in_=dq8[b].rearrange(
                    "(c p) d -> p c d", p=P))
            for hf in range(2):
                nc.sync.dma_start(out=h16[b][:, 4 * hf:4 * hf + 4, :],
                                  in_=h16_src[hf])
        negm[b] = smalls.tile([P, NT], F32, tag=f"negm{b}", name=f"negm{b}")
        negh[b] = smalls.tile([P, 2], F32, tag=f"negh{b}", name=f"negh{b}")
        z_col[b] = smalls.tile([P, NT], F32, tag=f"z_col{b}", name=f"z_col{b}")
        r_col[b] = smalls.tile([P, NT], F32, tag=f"r_col{b}", name=f"r_col{b}")

    PIECES = [(0, 2), (2, 4), (4, 5), (5, 6)]

    def prep_math(b, j):
        for k in range(*PIECES[j]):
            nc.vector.tensor_add(uT[b][:, k, :], qTld[b][:, k, :],
                                 w1wT_t[:, k, :])
            nc.vector.tensor_scalar_mul(qT8[b][:, k, :], in0=qTld[b][:, k, :],
                                        scalar1=0.125)

    def phaseA(b, i):
        """Logit block: ps_a[s, 0:L] for s-tile i; per-half reduce_max."""
        s0 = i * P
        ps_a = psA.tile([P, L], F32, tag="ps_a", name="ps_a")
        for hx, (t0, t1) in enumerate(halves):
            nc.tensor.matmul(ps_a[:, t0:t1], ones1, w1b16[:, t0:t1],
                             start=True, stop=False)
            for k in range(KD):
                nc.tensor.matmul(ps_a[:, t0:t1], hTt[b][:, k, s0:s0 + P],
                                 uT[b][:, k, t0:t1], start=False, stop=False)
            for g in range(KD // 2):
                nc.tensor.matmul(
                    ps_a[:, t0:t1], w2T[:, 2 * g:2 * g + 2, s0:s0 + P],
                    qT8[b][:, 2 * g:2 * g + 2, t0:t1],
                    start=False, stop=(g == KD // 2 - 1), perf_mode=DR,
                )
            # the half-0 reduce runs on DVE while half-1 matmuls stream
            nc.vector.reduce_max(negh[b][:, hx:hx + 1], ps_a[:, t0:t1],
                                 axis=AX, negate=True)
        nm = negm[b][:, i:i + 1]
        nc.vector.tensor_tensor(out=nm, in0=negh[b][:, 0:1],
                                in1=negh[b][:, 1:2], op=mybir.AluOpType.min)
        p16 = p16_p.tile([P, L], F16, tag="p16")
        nc.scalar.activation(out=p16, in_=ps_a, func=EXP, bias=nm,
                             scale=1.0, accum_out=z_col[b][:, i:i + 1])
        return p16

    def phaseT(b, i, p16):
        """PE-transpose p16 -> psum, Act copy psum -> pT8 (fp8, x PSCALE)."""
        ps_t = psTC.tile([P, 2 * L], F16, tag="ps_tc", name="ps_t")[:, 0:L]
        for tcn in range(NT):
            nc.tensor.transpose(ps_t[:, tcn * P:(tcn + 1) * P],
                                p16[:, tcn * P:(tcn + 1) * P], id128)
        pT8 = pT8_p.tile([P, NT, P], bdt, tag="pT8")
        scl = PSCALE if B_FP8 else 1.0
        nc.scalar.activation(out=pT8, in_=ps_t, func=COPY, scale=scl)
        if not (B_FP8 and RESID):
            return pT8, None
        pT8r = pT8_p.tile([P, NT, P], F8, tag="pT8r", name="pT8r")
        nc.scalar.activation(out=pT8r, in_=ps_t, func=COPY, scale=PSCALE / 16)
        return pT8, pT8r

    def phaseC(b, i, pT8, pT8r):
        """c-matmul (fp8 DR) + epilogue: cc = [c, h*c] fp16, store pair."""
        s0 = i * P
        ps_c = psTC.tile([P, L], F32, tag="ps_tc", name="ps_c")
        if B_FP8:
            groups = [(pT8, q8t[b], False)]
            if RESID:
                groups = [(pT8, q8t[b], False), (pT8r, dq8t[b], True)]
            for pr_t, qv, last in groups:
                for g in range(NT // 2):
                    pr = pr_t[:, 2 * g:2 * g + 2, :]
                    st = (pr_t is pT8 and g == 0)
                    sp = ((last or not RESID) and g == NT // 2 - 1)
                    if WIDE_C:
                        nc.tensor.matmul(ps_c[:, 0:D], pr,
                                         qv[:, 2 * g:2 * g + 2, :],
                                         start=st, stop=sp, perf_mode=DR)
                    else:
                        nc.tensor.matmul(ps_c[:, 0:512], pr,
                                         qv[:, 2 * g:2 * g + 2, 0:512],
                                         start=st, stop=sp, perf_mode=DR)
                        nc.tensor.matmul(ps_c[:, 512:D], pr,
                                         qv[:, 2 * g:2 * g + 2, 512:D],
                                         start=st, stop=sp, perf_mode=DR)
        else:
            for tcn in range(NT):
                lp = pT8[:, tcn, :]
                nc.tensor.matmul(ps_c[:, 0:512], lp, q8t[b][:, tcn, 0:512],
                                 start=(tcn == 0), stop=(tcn == NT - 1))
                nc.tensor.matmul(ps_c[:, 512:D], lp, q8t[b][:, tcn, 512:D],
                                 start=(tcn == 0), stop=(tcn == NT - 1))
        r_i = r_col[b][:, i:i + 1]
        nc.vector.reciprocal(r_i, z_col[b][:, i:i + 1])
        if B_FP8:
            nc.vector.tensor_scalar_mul(r_i, in0=r_i, scalar1=1.0 / PSCALE)
        cc = c16_p.tile([P, 2, D], F16, tag="cc", bufs=8, name=f"cc_{b}_{i}")
        nc.scalar.activation(out=cc[:, 0, :], in_=ps_c[:, 0:D], func=COPY,
                             scale=r_i)
        nc.vector.tensor_mul(cc[:, 1, :], h16[b][:, i, :], cc[:, 0, :])
        nc.gpsimd.dma_start(out=out[b, s0:s0 + P, 0:2 * D], in_=cc)
        return cc

    def phaseP2(b):
        """p2 = softmax over the 1024 row maxes, fully on-chip."""
        m_true = smalls.tile([P, NT], F32, tag=f"m_true{b}", name=f"m_true{b}")
        nc.vector.tensor_sub(m_true, w2b_col, negm[b])
        mloc = smalls.tile([P, 1], F32, tag=f"mloc{b}", name=f"mloc{b}")
        nc.vector.reduce_max(mloc, m_true, axis=AX)
        mm = smalls.tile([P, 1], F32, tag=f"mm{b}", name=f"mm{b}")
        nc.gpsimd.partition_all_reduce(mm, mloc, channels=P,
                                       reduce_op=bass_isa.ReduceOp.max)
        nmm = smalls.tile([P, 1], F32, tag=f"nmm{b}", name=f"nmm{b}")
        nc.vector.tensor_scalar_mul(nmm, in0=mm, scalar1=-1.0)
        e2 = smalls.tile([P, NT], F32, tag=f"e2{b}", name=f"e2{b}")
        zloc = smalls.tile([P, 1], F32, tag=f"zloc{b}", name=f"zloc{b}")
        nc.scalar.activation(out=e2, in_=m_true, func=EXP, bias=nmm,
                             scale=1.0, accum_out=zloc)
        z2 = smalls.tile([P, 1], F32, tag=f"z2{b}", name=f"z2{b}")
        nc.gpsimd.partition_all_reduce(z2, zloc, channels=P,
                                       reduce_op=bass_isa.ReduceOp.add)
        r2 = smalls.tile([P, 1], F32, tag=f"r2{b}", name=f"r2{b}")
        nc.vector.reciprocal(r2, z2)
        p2c[b] = smalls.tile([P, NT], F32, tag=f"p2c{b}", name=f"p2c{b}")
        nc.vector.tensor_scalar_mul(p2c[b], in0=e2, scalar1=r2)

    def qcc_one(b, i, cc):
        s0 = i * P
        qcc16 = ep_p.tile([P, D], F16, tag="qcc16", bufs=4)
        nc.vector.tensor_scalar_mul(qcc16, in0=cc[:, 1, :],
                                    scalar1=p2c[b][:, i:i + 1])
        eng = nc.sync if i % 2 == 0 else nc.gpsimd
        eng.dma_start(out=out[b, s0:s0 + P, 2 * D:3 * D], in_=qcc16)

    # ---- software pipeline: step s runs A(s), C(s-2), T(s-1) ----
    p16_t = {}
    pT8_t = {}
    cc_t = {}
    qcc_done = [0, 0]
    prep_loads(0)
    for j in range(4):
        prep_math(0, j)
    for s in range(NSTEP + 2):
        if s < NSTEP:
            b, i = divmod(s, NT)
            if s == 3:
                prep_loads(1)
            p16_t[s] = phaseA(b, i)
            if 3 <= s <= 6:
                prep_math(1, s - 3)  # after this step's reduce in DVE order
            if i == NT - 1:
                phaseP2(b)  # needs only the 8 reduce_maxes of batch b
        if s - 2 >= 0:
            bb, ii = divmod(s - 2, NT)
            cc_t[s - 2] = phaseC(bb, ii, *pT8_t.pop(s - 2))
            if s >= NT * bb + NT - 1:  # p2(bb) emitted at that step's A
                while qcc_done[bb] <= ii:
                    jj = qcc_done[bb]
                    qcc_one(bb, jj, cc_t[NT * bb + jj])
                    qcc_done[bb] += 1
        if 0 <= s - 1 < NSTEP:
            bb, ii = divmod(s - 1, NT)
            pT8_t[s - 1] = phaseT(bb, ii, p16_t.pop(s - 1))


def build():
    nc = bacc.Bacc()
    h = nc.dram_tensor("h", [BL, L, D], F16, kind="ExternalInput")
    hT = nc.dram_tensor("hT", [BL, D, L], F16, kind="ExternalInput")
    qT = nc.dram_tensor("qT", [BL, D, L], F16, kind="ExternalInput")
    bdt = F8 if B_FP8 else F16
    q8 = nc.dram_tensor("q8", [BL, L, D], bdt, kind="ExternalInput")
    dq8 = nc.dram_tensor("dq8", [BL, L, D], F8, kind="ExternalInput")
    w1wT = nc.dram_tensor("w1wT", [D, L], F16, kind="ExternalInput")
    w1b = nc.dram_tensor("w1_b", [L], F32, kind="ExternalInput")
    w2t = nc.dram_tensor("w2t", [D, L], F8, kind="ExternalInput")
    w2b = nc.dram_tensor("w2_b", [L], F32, kind="ExternalInput")
    out = nc.dram_tensor("out", [BL, L, 3 * D], F16, kind="ExternalOutput")
    with tile.TileContext(nc) as tc, ExitStack() as ctx:
        _emit(ctx, tc, (h[:], hT[:], qT[:], q8[:], dq8[:], w1wT[:], w1b[:],
                        w2t[:], w2b[:], out[:]))
    nc.compile()
    return nc


def _in_maps(inputs):
    import ml_dtypes

    h16 = np.asarray(inputs["h"], np.float16)
    hT = np.ascontiguousarray(np.swapaxes(h16, 1, 2))
    q16 = np.asarray(inputs["q"], np.float16)
    qT = np.ascontiguousarray(np.swapaxes(q16, 1, 2))
    if B_FP8:
        q8 = np.ascontiguousarray(q16.astype(ml_dtypes.float8_e4m3fn))
        dq = q16.astype(np.float32) - q8.astype(np.float32)
        dq8 = np.ascontiguousarray((16.0 * dq).astype(ml_dtypes.float8_e4m3fn))
    else:
        q8 = np.ascontiguousarray(q16)
        dq8 = np.zeros_like(q16, dtype=ml_dtypes.float8_e4m3fn)
    w1wT = np.ascontiguousarray(np.asarray(inputs["w1_w"], np.float16).T)
    w1b = np.ascontiguousarray(np.asarray(inputs["w1_b"], np.float32))
    w2b = np.ascontiguousarray(np.asarray(inputs["w2_b"], np.float32))
    w2wT = np.asarray(inputs["w2_w"], np.float32).T  # [D, L]
    w2t = np.ascontiguousarray((8.0 * w2wT)).astype(ml_dtypes.float8_e4m3fn)
    maps = []
    for c in range(NCORES):
        sl = slice(c * BL, (c + 1) * BL)
        maps.append({
            "h": np.ascontiguousarray(h16[sl]),
            "hT": np.ascontiguousarray(hT[sl]),
            "qT": np.ascontiguousarray(qT[sl]),
            "q8": np.ascontiguousarray(q8[sl]),
            "dq8": np.ascontiguousarray(dq8[sl]),
            "w1wT": w1wT, "w1_b": w1b, "w2t": w2t, "w2_b": w2b,
        })
    return maps


def _assemble(inputs, results):
    full = np.empty((B, L, 4 * D), np.float32)
    full[:, :, 0:D] = np.asarray(inputs["h"], np.float32)
    dev = np.concatenate([np.asarray(r["out"], np.float32) for r in results],
                         axis=0)
    full[:, :, D:4 * D] = dev
    return full


def kernel(**inputs):
    nc = build()
    res = run_bass_kernel_spmd(nc, _in_maps(inputs), core_ids=list(range(NCORES)))
    return _assemble(inputs, res.results)


def run_profiled(inputs, **kwargs):
    nc = build()
    res = run_bass_kernel_spmd(
        nc, _in_maps(inputs), core_ids=list(range(NCORES)), trace=True, **kwargs
    )
    return _assemble(inputs, res.results), res


# revision 26
# speedup vs baseline: 1.0602x; 1.0602x over previous
"""BiDAF2 attention kernel for Trainium2, 8-core data parallel over batch.

reference (per batch b):
  w1h[s,l] = h[s,:] @ w1_w[l,:] + w1_b[l]
  w2q[t,l] = q[t,:] @ w2_w[l,:] + w2_b[l]
  a[s,t]   = w1h[s,t] + w2q[t,s] + h[s,:]@q[t,:]
  p        = softmax_t(a);  c[s,:] = p[s,:] @ q
  m[s]     = max_t a[s,t];  p2 = softmax_s(m)
  out      = concat([h, c, h*c, (h*p2)*c], axis=-1)

v9 design (cost-model driven; single-core TimelineSim matches HW within 4%):
  - PE p-state: the tensor engine only reaches 2.4 GHz after 3us of
    CONTINUOUS execution, so the kernel is one long software pipeline:
    step s in [0,16): A(s) logit block, T(s-1) PE-transpose of p (identity
    matmul -- no DMA transposes anywhere), C(s-2) fp8 c-matmul.
  - Host prep (sharding time) ships layout/precision variants only:
    h16 [t,d] f16, hT [d,t] f16, qT [d,t] f16, q8 [t,d] fp8e4m3,
    dq8 = fp8(q16 - q8) [t,d], w1wT [d,l] f16, w2T = 8*w2^T fp8 (the
    8x / 0.125 scales bake out). uT = qT + w1wT is a DVE add on device;
    qT8 = qT * 0.125 (fp8 cast).
  - Logits: a = hT^T@uT (fp16) + w2T^T@qT8 (fp8 DoubleRow, 2x PE rate)
    + w1_b via ones-row matmul. Per-half reduce_max overlaps the second
    half-block; softmax row-sum fused into the (half-split) exp.
  - p transposed by PE into a PSUM pool shared with the c-accumulator
    (2+2 banks + 2x2 psA = 8 banks exactly); Act copies psum -> pT8 fp8
    with scale 64 (keeps small p out of fp8 subnormals); 1/64 is folded
    into the c epilogue scale.
  - c = pT8^T@q8 + pT8^T@dq8 in fp8 DoubleRow, both groups at psum scale
    64: the residual term cancels the fp8 quantization of q (hc section
    err 0.83 -> 0.26 offline) at half the PE cost of an fp16 c-matmul.
  - p2 = softmax over the 1024 row maxes entirely on-chip via gpsimd
    partition_all_reduce (max, then add) -- no DRAM roundtrip.
  - All loads ride the sync(SP) queue in explicit first-use priority
    order (the serial DMA device follows queue order; SWDGE prefetches
    would otherwise starve the critical path); scalar(Act) issues no
    DMAs so exp/copyT/ccCopy never queue behind a DMA trigger.
  - Output is fp16 [c, h*c, (h*c)*p2]; section 0 (h verbatim) is filled
    host-side from the exact f32 input.
"""

import os
import sys

for _p in ("/opt/trn_rl_repo", "/root/.axon_site/_ro/trn_rl_repo"):
    if os.path.isdir(_p) and _p not in sys.path:
        sys.path.append(_p)

from contextlib import ExitStack

import numpy as np

import concourse.bass as bass
import concourse.tile as tile
from concourse import bacc, bass_isa, mybir
from concourse.bass_utils import run_bass_kernel_spmd

B, L, D = 16, 1024, 768
NCORES = 8
BL = B // NCORES  # batches per core
P = 128
KD = D // P  # 6 d-chunks
NT = L // P  # 8 t-chunks == 8 s-tiles
NSTEP = BL * NT  # 16 pipeline steps
F16 = mybir.dt.float16
F32 = mybir.dt.float32
F8 = mybir.dt.float8e4
EXP = mybir.ActivationFunctionType.Exp
COPY = mybir.ActivationFunctionType.Copy
AX = mybir.AxisListType.X
DR = mybir.MatmulPerfMode.DoubleRow

B_FP8 = True  # phaseB (c = p@q) via fp8 DoubleRow; False = fp16 fallback
RESID = True  # add a scale-4 residual DR pass (p (x) 16*(q - q8)) for accuracy
WIDE_A = False  # single 1024-col A matmuls (PSUM bank-crossing; HW-verify!)
WIDE_C = False  # single 768-col C matmuls
PSCALE = 64.0  # pT8 pre-scale (folded out in the c epilogue)
REPEAT = 1  # benchmarking aid: run the whole body REPEAT times via For_i


def _emit(ctx: ExitStack, tc: tile.TileContext, aps):
    if REPEAT > 1:
        with tc.For_i(0, REPEAT, 1):
            _emit_once(ctx, tc, aps)
    else:
        _emit_once(ctx, tc, aps)


def _emit_once(ctx: ExitStack, tc: tile.TileContext, aps):
    nc = tc.nc
    h, hT, qT, q8, dq8, w1wT, w1b, w2t, w2b, out = aps
    halves = [(0, L)] if WIDE_A else [(0, 512), (512, 1024)]

    const = ctx.enter_context(tc.tile_pool(name="const", bufs=1))
    qTld_p = ctx.enter_context(tc.tile_pool(name="qTld", bufs=2))
    uT_p = ctx.enter_context(tc.tile_pool(name="uT", bufs=2))
    qT8_p = ctx.enter_context(tc.tile_pool(name="qT8", bufs=2))
    hT_p = ctx.enter_context(tc.tile_pool(name="hTt", bufs=2))
    h16_p = ctx.enter_context(tc.tile_pool(name="h16", bufs=2))
    q8_p = ctx.enter_context(tc.tile_pool(name="q8t", bufs=2))
    p16_p = ctx.enter_context(tc.tile_pool(name="p16", bufs=2))
    pT8_p = ctx.enter_context(tc.tile_pool(name="pT8", bufs=4))
    c16_p = ctx.enter_context(tc.tile_pool(name="c16", bufs=8))
    ep_p = ctx.enter_context(tc.tile_pool(name="ep", bufs=4))
    smalls = ctx.enter_context(tc.tile_pool(name="smalls", bufs=1))
    psA = ctx.enter_context(tc.tile_pool(name="psA", bufs=2, space="PSUM"))
    psTC = ctx.enter_context(tc.tile_pool(name="psTC", bufs=2, space="PSUM"))

    # ---- constants ----
    ones1 = const.tile([1, P], F16)
    nc.vector.memset(ones1, 1.0)
    id128 = const.tile([P, P], F16, name="id128")
    nc.vector.memset(id128, 1.0)
    # identity: keep the 1.0 where col == partition (iota = col - p == 0)
    nc.gpsimd.affine_select(
        out=id128, in_=id128, pattern=[[1, P]], base=0, channel_multiplier=-1,
        compare_op=mybir.AluOpType.is_equal, fill=0.0,
    )
    w1b16 = const.tile([1, L], F16)
    nc.gpsimd.dma_start(out=w1b16, in_=w1b[None, :])
    w2b_col = const.tile([P, NT], F32)
    nc.gpsimd.dma_start(out=w2b_col, in_=w2b.rearrange("(c p) -> p c", p=P))
    # Load-queue discipline: ALL latency-critical loads go on sync (SP) in
    # priority order; scalar (Act) issues no DMAs so the exp/copyT/ccCopy
    # chain never stalls behind a 667ns DMA trigger; gpsimd (SWDGE) carries
    # the bulk h16/q8 loads and the cc stores.
    w1wT_t = const.tile([P, KD, L], F16, name="w1wTt")  # [d_part, kd, l]
    w2T = const.tile([P, KD, L], F8, name="w2T")  # [d_part, kd, l]
    w2T_src = w2t.rearrange("(k p) l -> p k l", p=P)

    bdt = F8 if B_FP8 else F16
    uT = {}
    qT8 = {}
    qTld = {}
    dq8t = {}
    hTt = {}
    h16 = {}
    q8t = {}
    negm = {}
    negh = {}
    z_col = {}
    zh_col = {}
    r_col = {}
    p2c = {}

    def prep_loads(b):
        qTld[b] = qTld_p.tile([P, KD, L], F16, tag="qTld", name=f"qTld_{b}")
        uT[b] = uT_p.tile([P, KD, L], F16, tag="uT", name=f"uT_{b}")
        qT8[b] = qT8_p.tile([P, KD, L], F8, tag="qT8", name=f"qT8_{b}")
        hTt[b] = hT_p.tile([P, KD, L], F16, tag="hTt", name=f"hTt_{b}")
        hT_src = hT[b].rearrange("(k p) l -> p k l", p=P)
        h16[b] = h16_p.tile([P, NT, D], F16, tag="h16", name=f"h16_{b}")
        q8t[b] = q8_p.tile([P, NT, D], bdt, tag="q8t", name=f"q8t_{b}")
        if B_FP8 and RESID:
            dq8t[b] = q8_p.tile([P, NT, D], F8, tag="dq8t", name=f"dq8t_{b}")
        h16_src = [h[b, 512 * hf:512 * hf + 512, :].rearrange(
            "(c p) d -> p c d", p=P) for hf in range(2)]
        if b == 0:
            # single-queue priority order. The A(0) stationaries (hT quarter
            # 0, w2T half 0) go FIRST so the PE can chew each uT chunk as the
            # k-loads stream in; then everything else in first-use order.
            nc.sync.dma_start(out=w1wT_t[:, 0, :], in_=w1wT[0:P, :])
            nc.sync.dma_start(out=qTld[b][:, 0, :], in_=qT[b, 0:P, :])
            nc.sync.dma_start(out=hTt[b][:, :, 0:256], in_=hT_src[:, :, 0:256])
            nc.sync.dma_start(out=w2T[:, :, 0:512], in_=w2T_src[:, :, 0:512])
            for k in range(1, KD):
                r = slice(k * P, (k + 1) * P)
                nc.sync.dma_start(out=w1wT_t[:, k, :], in_=w1wT[r, :])
                nc.sync.dma_start(out=qTld[b][:, k, :], in_=qT[b, r, :])
            nc.sync.dma_start(out=hTt[b][:, :, 256:512],
                              in_=hT_src[:, :, 256:512])
            nc.sync.dma_start(out=q8t[b], in_=q8[b].rearrange(
                "(c p) d -> p c d", p=P))
            if B_FP8 and RESID:
                nc.sync.dma_start(out=dq8t[b], in_=dq8[b].rearrange(
                    "(c p) d -> p c d", p=P))
            nc.sync.dma_start(out=hTt[b][:, :, 512:768],
                              in_=hT_src[:, :, 512:768])
            nc.sync.dma_start(out=h16[b][:, 0:4, :], in_=h16_src[0])
            nc.sync.dma_start(out=hTt[b][:, :, 768:L], in_=hT_src[:, :, 768:L])
            nc.sync.dma_start(out=w2T[:, :, 512:L], in_=w2T_src[:, :, 512:L])
            nc.sync.dma_start(out=h16[b][:, 4:8, :], in_=h16_src[1])
        else:
            for k in range(KD):
                r = slice(k * P, (k + 1) * P)
                nc.sync.dma_start(out=qTld[b][:, k, :], in_=qT[b, r, :])
            nc.sync.dma_start(out=hTt[b], in_=hT_src)
            nc.sync.dma_start(out=q8t[b], in_=q8[b].rearrange(
                "(c p) d -> p c d", p=P))
            if B_FP8 and RESID:
                nc.sync.dma_start(out=dq8t[b], in_=dq8[b].rearrange(
                    "(c p) d -> p c d", p=P))
            for hf in range(2):
                nc.sync.dma_start(out=h16[b][:, 4 * hf:4 * hf + 4, :],
                                  in_=h16_src[hf])
        negm[b] = smalls.tile([P, NT], F32, tag=f"negm{b}", name=f"negm{b}")
        negh[b] = smalls.tile([P, 2], F32, tag=f"negh{b}", name=f"negh{b}")
        z_col[b] = smalls.tile([P, NT], F32, tag=f"z_col{b}", name=f"z_col{b}")
        zh_col[b] = smalls.tile([P, 2 * NT], F32, tag=f"zh_col{b}",
                                name=f"zh_col{b}")
        r_col[b] = smalls.tile([P, NT], F32, tag=f"r_col{b}", name=f"r_col{b}")

    PIECES = [(0, 2), (2, 4), (4, 5), (5, 6)]

    def prep_math(b, j):
        # adds first (each unblocks an A-stream fp16 matmul); fp8 casts after
        # (only the half-end DR matmuls need them)
        for k in range(*PIECES[j]):
            nc.vector.tensor_add(uT[b][:, k, :], qTld[b][:, k, :],
                                 w1wT_t[:, k, :])
        for k in range(*PIECES[j]):
            nc.vector.tensor_scalar_mul(qT8[b][:, k, :], in0=qTld[b][:, k, :],
                                        scalar1=0.125)

    def phaseA(b, i):
        """Logit block: ps_a[s, 0:L] for s-tile i; per-half reduce_max."""
        s0 = i * P
        ps_a = psA.tile([P, L], F32, tag="ps_a", name="ps_a")
        for hx, (t0, t1) in enumerate(halves):
            nc.tensor.matmul(ps_a[:, t0:t1], ones1, w1b16[:, t0:t1],
                             start=True, stop=False)
            for k in range(KD):
                nc.tensor.matmul(ps_a[:, t0:t1], hTt[b][:, k, s0:s0 + P],
                                 uT[b][:, k, t0:t1], start=False, stop=False)
            for g in range(KD // 2):
                nc.tensor.matmul(
                    ps_a[:, t0:t1], w2T[:, 2 * g:2 * g + 2, s0:s0 + P],
                    qT8[b][:, 2 * g:2 * g + 2, t0:t1],
                    start=False, stop=(g == KD // 2 - 1), perf_mode=DR,
                )
            # the half-0 reduce runs on DVE while half-1 matmuls stream
            nc.vector.reduce_max(negh[b][:, hx:hx + 1], ps_a[:, t0:t1],
                                 axis=AX, negate=True)
        nm = negm[b][:, i:i + 1]
        nc.vector.tensor_tensor(out=nm, in0=negh[b][:, 0:1],
                                in1=negh[b][:, 1:2], op=mybir.AluOpType.min)
        p16 = p16_p.tile([P, L], F16, tag="p16")
        for hx, (t0, t1) in enumerate(
                [(0, 512), (512, L)] if not WIDE_A else [(0, L)]):
            nc.scalar.activation(out=p16[:, t0:t1], in_=ps_a[:, t0:t1],
                                 func=EXP, bias=nm, scale=1.0,
                                 accum_out=zh_col[b][:, 2 * i + hx:2 * i + hx + 1])
        return p16

    def phaseT(b, i, p16):
        """PE-transpose p16 -> psum, Act copy psum -> pT8 (fp8, x PSCALE)."""
        ps_t = psTC.tile([P, 2 * L], F16, tag="ps_tc", name="ps_t")[:, 0:L]
        for tcn in range(NT):
            nc.tensor.transpose(ps_t[:, tcn * P:(tcn + 1) * P],
                                p16[:, tcn * P:(tcn + 1) * P], id128)
        pT8 = pT8_p.tile([P, NT, P], bdt, tag="pT8")
        scl = PSCALE if B_FP8 else 1.0
        nc.scalar.activation(out=pT8, in_=ps_t, func=COPY, scale=scl)
        return pT8, pT8

    def phaseC(b, i, pT8, pT8r):
        """c-matmul (fp8 DR) + epilogue: cc = [c, h*c] fp16, store pair."""
        s0 = i * P
        ps_c = psTC.tile([P, L], F32, tag="ps_tc", name="ps_c")
        if B_FP8:
            groups = [(pT8, q8t[b], False)]
            if RESID:
                groups = [(pT8, q8t[b], False), (pT8, dq8t[b], True)]
            for gi, (pr_t, qv, last) in enumerate(groups):
                for g in range(NT // 2):
                    pr = pr_t[:, 2 * g:2 * g + 2, :]
                    st = (gi == 0 and g == 0)
                    sp = ((last or not RESID) and g == NT // 2 - 1)
                    if WIDE_C:
                        nc.tensor.matmul(ps_c[:, 0:D], pr,
                                         qv[:, 2 * g:2 * g + 2, :],
                                         start=st, stop=sp, perf_mode=DR)
                    else:
                        nc.tensor.matmul(ps_c[:, 0:512], pr,
                                         qv[:, 2 * g:2 * g + 2, 0:512],
                                         start=st, stop=sp, perf_mode=DR)
                        nc.tensor.matmul(ps_c[:, 512:D], pr,
                                         qv[:, 2 * g:2 * g + 2, 512:D],
                                         start=st, stop=sp, perf_mode=DR)
        else:
            for tcn in range(NT):
                lp = pT8[:, tcn, :]
                nc.tensor.matmul(ps_c[:, 0:512], lp, q8t[b][:, tcn, 0:512],
                                 start=(tcn == 0), stop=(tcn == NT - 1))
                nc.tensor.matmul(ps_c[:, 512:D], lp, q8t[b][:, tcn, 512:D],
                                 start=(tcn == 0), stop=(tcn == NT - 1))
        r_i = r_col[b][:, i:i + 1]
        z_i = z_col[b][:, i:i + 1]
        nc.vector.tensor_add(z_i, zh_col[b][:, 2 * i:2 * i + 1],
                             zh_col[b][:, 2 * i + 1:2 * i + 2])
        nc.vector.reciprocal(r_i, z_i)
        if B_FP8:
            nc.vector.tensor_scalar_mul(r_i, in0=r_i, scalar1=1.0 / PSCALE)
        cc = c16_p.tile([P, 2, D], F16, tag="cc", bufs=8, name=f"cc_{b}_{i}")
        nc.scalar.activation(out=cc[:, 0, :], in_=ps_c[:, 0:D], func=COPY,
                             scale=r_i)
        nc.vector.tensor_mul(cc[:, 1, :], h16[b][:, i, :], cc[:, 0, :])
        nc.gpsimd.dma_start(out=out[b, s0:s0 + P, 0:2 * D], in_=cc)
        return cc

    def phaseP2(b):
        """p2 = softmax over the 1024 row maxes, fully on-chip."""
        m_true = smalls.tile([P, NT], F32, tag=f"m_true{b}", name=f"m_true{b}")
        nc.vector.tensor_sub(m_true, w2b_col, negm[b])
        mloc = smalls.tile([P, 1], F32, tag=f"mloc{b}", name=f"mloc{b}")
        nc.vector.reduce_max(mloc, m_true, axis=AX)
        mm = smalls.tile([P, 1], F32, tag=f"mm{b}", name=f"mm{b}")
        nc.gpsimd.partition_all_reduce(mm, mloc, channels=P,
                                       reduce_op=bass_isa.ReduceOp.max)
        nmm = smalls.tile([P, 1], F32, tag=f"nmm{b}", name=f"nmm{b}")
        nc.vector.tensor_scalar_mul(nmm, in0=mm, scalar1=-1.0)
        e2 = smalls.tile([P, NT], F32, tag=f"e2{b}", name=f"e2{b}")
        zloc = smalls.tile([P, 1], F32, tag=f"zloc{b}", name=f"zloc{b}")
        nc.scalar.activation(out=e2, in_=m_true, func=EXP, bias=nmm,
                             scale=1.0, accum_out=zloc)
        z2 = smalls.tile([P, 1], F32, tag=f"z2{b}", name=f"z2{b}")
        nc.gpsimd.partition_all_reduce(z2, zloc, channels=P,
                                       reduce_op=bass_isa.ReduceOp.add)
        r2 = smalls.tile([P, 1], F32, tag=f"r2{b}", name=f"r2{b}")
        nc.vector.reciprocal(r2, z2)
        p2c[b] = smalls.tile([P, NT], F32, tag=f"p2c{b}", name=f"p2c{b}")
        nc.vector.tensor_scalar_mul(p2c[b], in0=e2, scalar1=r2)

    def qcc_one(b, i, cc):
        s0 = i * P
        qcc16 = ep_p.tile([P, D], F16, tag="qcc16", bufs=4)
        nc.vector.tensor_scalar_mul(qcc16, in0=cc[:, 1, :],
                                    scalar1=p2c[b][:, i:i + 1])
        eng = nc.sync if i % 2 == 0 else nc.gpsimd
        eng.dma_start(out=out[b, s0:s0 + P, 2 * D:3 * D], in_=qcc16)

    # ---- software pipeline: step s runs A(s), C(s-2), T(s-1) ----
    p16_t = {}
    pT8_t = {}
    cc_t = {}
    qcc_done = [0, 0]
    prep_loads(0)
    for j in range(4):
        prep_math(0, j)
    for s in range(NSTEP + 2):
        if s < NSTEP:
            b, i = divmod(s, NT)
            if s == 3:
                prep_loads(1)
            p16_t[s] = phaseA(b, i)
            if 3 <= s <= 6:
                prep_math(1, s - 3)  # after this step's reduce in DVE order
            if i == NT - 1:
                phaseP2(b)  # needs only the 8 reduce_maxes of batch b
        if 0 <= s - 1 < NSTEP:
            bb, ii = divmod(s - 1, NT)
            pT8_t[s - 1] = phaseT(bb, ii, p16_t.pop(s - 1))
        if s - 2 >= 0:
            bb, ii = divmod(s - 2, NT)
            cc_t[s - 2] = phaseC(bb, ii, *pT8_t.pop(s - 2))
            if s >= NT * bb + NT - 1:  # p2(bb) emitted at that step's A
                while qcc_done[bb] <= ii:
                    jj = qcc_done[bb]
                    qcc_one(bb, jj, cc_t[NT * bb + jj])
                    qcc_done[bb] += 1


def build():
    nc = bacc.Bacc()
    h = nc.dram_tensor("h", [BL, L, D], F16, kind="ExternalInput")
    hT = nc.dram_tensor("hT", [BL, D, L], F16, kind="ExternalInput")
    qT = nc.dram_tensor("qT", [BL, D, L], F16, kind="ExternalInput")
    bdt = F8 if B_FP8 else F16
    q8 = nc.dram_tensor("q8", [BL, L, D], bdt, kind="ExternalInput")
    dq8 = nc.dram_tensor("dq8", [BL, L, D], F8, kind="ExternalInput")
    w1wT = nc.dram_tensor("w1wT", [D, L], F16, kind="ExternalInput")
    w1b = nc.dram_tensor("w1_b", [L], F32, kind="ExternalInput")
    w2t = nc.dram_tensor("w2t", [D, L], F8, kind="ExternalInput")
    w2b = nc.dram_tensor("w2_b", [L], F32, kind="ExternalInput")
    out = nc.dram_tensor("out", [BL, L, 3 * D], F16, kind="ExternalOutput")
    with tile.TileContext(nc) as tc, ExitStack() as ctx:
        _emit(ctx, tc, (h[:], hT[:], qT[:], q8[:], dq8[:], w1wT[:], w1b[:],
                        w2t[:], w2b[:], out[:]))
    nc.compile()
    return nc


def _in_maps(inputs):
    import ml_dtypes

    h16 = np.asarray(inputs["h"], np.float16)
    hT = np.ascontiguousarray(np.swapaxes(h16, 1, 2))
    q16 = np.asarray(inputs["q"], np.float16)
    qT = np.ascontiguousarray(np.swapaxes(q16, 1, 2))
    if B_FP8:
        q8 = np.ascontiguousarray(q16.astype(ml_dtypes.float8_e4m3fn))
        dq = q16.astype(np.float32) - q8.astype(np.float32)
        dq8 = np.ascontiguousarray(dq.astype(ml_dtypes.float8_e4m3fn))
    else:
        q8 = np.ascontiguousarray(q16)
        dq8 = np.zeros_like(q16, dtype=ml_dtypes.float8_e4m3fn)
    w1wT = np.ascontiguousarray(np.asarray(inputs["w1_w"], np.float16).T)
    w1b = np.ascontiguousarray(np.asarray(inputs["w1_b"], np.float32))
    w2b = np.ascontiguousarray(np.asarray(inputs["w2_b"], np.float32))
    w2wT = np.asarray(inputs["w2_w"], np.float32).T  # [D, L]
    w2t = np.ascontiguousarray((8.0 * w2wT)).astype(ml_dtypes.float8_e4m3fn)
    maps = []
    for c in range(NCORES):
        sl = slice(c * BL, (c + 1) * BL)
        maps.append({
            "h": np.ascontiguousarray(h16[sl]),
            "hT": np.ascontiguousarray(hT[sl]),
            "qT": np.ascontiguousarray(qT[sl]),
            "q8": np.ascontiguousarray(q8[sl]),
            "dq8": np.ascontiguousarray(dq8[sl]),
            "w1wT": w1wT, "w1_b": w1b, "w2t": w2t, "w2_b": w2b,
        })
    return maps


def _assemble(inputs, results):
    full = np.empty((B, L, 4 * D), np.float32)
    full[:, :, 0:D] = np.asarray(inputs["h"], np.float32)
    dev = np.concatenate([np.asarray(r["out"], np.float32) for r in results],
                         axis=0)
    full[:, :, D:4 * D] = dev
    return full


def kernel(**inputs):
    nc = build()
    res = run_bass_kernel_spmd(nc, _in_maps(inputs), core_ids=list(range(NCORES)))
    return _assemble(inputs, res.results)


def run_profiled(inputs, **kwargs):
    nc = build()
    res = run_bass_kernel_spmd(
        nc, _in_maps(inputs), core_ids=list(range(NCORES)), trace=True, **kwargs
    )
    return _assemble(inputs, res.results), res
